# revision 49
# baseline (speedup 1.0000x reference)
"""Multi-head attention (B=2, S=2048, D=1024, H=16, DK=64) on 8 Trainium2 cores.

Sharding: 8 cores x (1 batch, 4 heads) each.  Core c handles batch c//4,
heads [4*(c%4) : 4*(c%4)+4].  Each core computes its heads' slice of the
output projection (rows of Wo for its heads); the host sums the 4 partial
outputs per batch (fp32) and adds the bias.

Per-core dataflow (all matmuls bf16, PSUM fp32):
  - projections as in the classic layout: qhT/khT [128=head-pair, S] via PE;
    swap tiles (SBUF->SBUF DMA) duplicate each head's 64 partitions into the
    opposite half so scores can use PE ROW TILING: the two 512-wide j-halves
    of every scores chunk run CONCURRENTLY on sub-arrays T0 (partitions
    0-63) and T8 (64-127), halving effective scores time at K=64.
  - exp split across TWO engines per m-chunk: j0 half on ACT (table exp),
    j1 half on a custom fused DVE op (1 + w(c0 + c1 w))^16, w = z/128,
    equal throughput, both writing the same bf16 attnT tile.
  - scores psum is [128,512] (1 bank) double-buffered per engine; pout
    [65,1024] double-buffered; one [128,1024] fill slot => exactly 8 banks.
  - v-projection runs as fill work (vT arrives 3rd in the DMA order); the
    first attention unit defers attn@v by 8 m-chunks to cover the vT DMA.
  - output projection bf16, staged bf16 (Pool engine) and summed on host.
"""

import numpy as np
import ml_dtypes
from contextlib import ExitStack

import concourse.bass as bass
import concourse.tile as tile
from concourse import bacc, mybir
from concourse import bass_utils
from concourse import dve_ops
from concourse.dve_spec import Spec, Src0, One, C0, C1, sq, lower
from concourse.dve_uop import DveOpSpec
from concourse.dve_table_gen import dve_ver_for

B, S, D, H, DK = 2, 2048, 1024, 16, 64
N_CORES = 8
HPC = 4            # heads per core
PAIRS = HPC // 2   # head pairs per core
KC = D // 128      # contraction chunks over D
MC = S // 128      # m (key) chunks
F32 = mybir.dt.float32
BF16 = mybir.dt.bfloat16
BF16_NP = ml_dtypes.bfloat16

_COMPILED = {}

# ---- custom DVE op: exp(z/8) = p(z/128)^16, p = 1 + w(c0 + c1 w) ----------


def _fit_exp_coeffs():
    w = np.linspace(-0.2, 0.2, 8001)
    a_mat = np.stack([w, w * w], axis=1)
    t = np.exp(w)
    rw = 1.0 / t  # minimize relative error
    coef, *_ = np.linalg.lstsq(a_mat * rw[:, None], (t - 1.0) * rw, rcond=None)
    return [float(c) for c in coef]


_A1, _A2 = _fit_exp_coeffs()
EXPC0 = _A1 / 128.0
EXPC1 = _A2 / (128.0 ** 2)


def _exp16_ref(in0, in1, c0, c1, c2):
    x = np.asarray(in0, np.float32)
    p = (1.0 + x * (c0 + x * c1)).astype(np.float32)
    for _ in range(4):
        p = (p * p).astype(np.float32)
    return p


def _register_exp16():
    name = "EXP16_ANT"
    if name in dve_ops._SUB_OPCODE_FOR_NAME:
        for op in dve_ops.OPS:
            if op.name == name:
                return op
    body = sq(sq(sq(sq(One + Src0 * (C0 + Src0 * C1)))))
    spec = Spec(body=body, reference=_exp16_ref)
    row = max(dve_ops._SUB_OPCODE_FOR_NAME.values()) + 1
    dve_ops._SUB_OPCODE_FOR_NAME[name] = row
    ver = dve_ver_for("TRN2")
    tmp = DveOpSpec(name=name, opcode=row, uops=lower(spec, ver=ver), rd1_en=False)
    op = dve_ops.DveOp(name, spec, subdim=False, uops_sha={ver: tmp.sha(ver)})
    dve_ops.OPS.append(op)
    dve_ops.CUSTOM_DVE_SPECS[name] = spec
    return op


EXP16 = _register_exp16()


def _emit(tc, qT, kT, vT, wq, wk, wv, wo, out_dram):
    nc = tc.nc
    AFT = mybir.ActivationFunctionType
    qTa, kTa, vTa = qT.ap(), kT.ap(), vT.ap()
    wqa, wka, wva, woa = wq.ap(), wk.ap(), wv.ap(), wo.ap()
    outa = out_dram.ap()

    with ExitStack() as ctx:
        big = ctx.enter_context(tc.tile_pool(name="big", bufs=1))
        att = ctx.enter_context(tc.tile_pool(name="att", bufs=12))
        dance = ctx.enter_context(tc.tile_pool(name="dance", bufs=1))
        ostage = ctx.enter_context(tc.tile_pool(name="ostage", bufs=4))
        # PSUM (8 banks): scores 2x[128,512] = 2, pout 2x[65,1024] = 4,
        # fill slot 1x[128,1024] = 2.
        ppool = ctx.enter_context(tc.tile_pool(name="psum", bufs=2, space="PSUM"))
        popool = ctx.enter_context(tc.tile_pool(name="psum_o", bufs=2, space="PSUM"))
        fpool = ctx.enter_context(tc.tile_pool(name="psum_f", bufs=1, space="PSUM"))

        # ---- weights first (small), then chunked inputs -------------------
        wq_sb = big.tile([128, KC, HPC * DK], BF16, tag="wq")
        wk_sb = big.tile([128, KC, HPC * DK], BF16, tag="wk")
        wv_sb = big.tile([128, KC, HPC * DK], BF16, tag="wv")
        wo_sb = big.tile([128, PAIRS, D], BF16, tag="wo")
        nc.sync.dma_start(wk_sb[:], wka.rearrange("(c p) n -> p c n", p=128))
        nc.sync.dma_start(wq_sb[:], wqa.rearrange("(c p) n -> p c n", p=128))

        # warm the ACT exp table during the DMA phase
        warm_sb = big.tile([1, 64], BF16, tag="warm")
        nc.vector.memset(warm_sb[:], 1.0)
        nc.scalar.activation(warm_sb[:], warm_sb[:], AFT.Exp)

        # DMA order: kT -> qT(qc0 half) -> vT -> qT(qc1 half) -> wv, wo.
        kT_sb = big.tile([128, KC, S], BF16, tag="kT")
        qT_sb = big.tile([128, KC, S], BF16, tag="qT")
        vT_sb = big.tile([128, KC, S], BF16, tag="vT")
        kTr = kTa.rearrange("(c p) s -> p c s", p=128)
        qTr = qTa.rearrange("(c p) s -> p c s", p=128)
        nc.sync.dma_start(kT_sb[:], kTr)
        nc.sync.dma_start(qT_sb[:, :, 0:1024], qTr[:, :, 0:1024])
        nc.sync.dma_start(wv_sb[:], wva.rearrange("(c p) n -> p c n", p=128))
        nc.sync.dma_start(vT_sb[:], vTa.rearrange("(c p) s -> p c s", p=128))
        nc.sync.dma_start(qT_sb[:, :, 1024:2048], qTr[:, :, 1024:2048])
        nc.sync.dma_start(wo_sb[:], woa.rearrange("(c p) d -> p c d", p=128))

        # vh with a ones column per (m-chunk, head): [128, MC, HPC, 65]
        vh_sb = big.tile([128, MC, HPC, DK + 1], BF16, tag="vh")
        nc.vector.memset(vh_sb[:], 1.0)

        # per-head qh/kh tiles with the OTHER head's partition half zeroed,
        # so scores contract over the full 128 partitions: every matmul in
        # the kernel is then (128,128)-mode -> no PE tiling-mode switches.
        qhp_sb = [big.tile([128, S], BF16, tag=f"qhp{h}", name=f"qhp{h}")
                  for h in range(HPC)]
        khp_sb = [big.tile([128, S], BF16, tag=f"khp{h}", name=f"khp{h}")
                  for h in range(HPC)]
        for h in range(HPC):
            nc.gpsimd.memset(qhp_sb[h][:], 0.0)
            nc.gpsimd.memset(khp_sb[h][:], 0.0)
        outT2_sb = [big.tile([128, S], BF16, tag=f"o2{p}", name=f"o2{p}")
                    for p in range(PAIRS)]

        def emit_proj_qk(p, w_sb, src, dsts, sc):
            pss = [ppool.tile([128, 512], F32, tag="pp", name=f"ps_proj{j}")
                   for j in range(2)]
            for kc in range(KC):
                for j in range(2):
                    nc.tensor.matmul(
                        pss[j][:],
                        w_sb[:, kc, p * 128:(p + 1) * 128],
                        src[:, kc, sc * 1024 + j * 512: sc * 1024 + (j + 1) * 512],
                        start=(kc == 0),
                        stop=(kc == KC - 1),
                    )
            for j in range(2):
                sl = slice(sc * 1024 + j * 512, sc * 1024 + (j + 1) * 512)
                nc.vector.tensor_copy(dsts[2 * p][0:64, sl], pss[j][0:64, :])
                nc.vector.tensor_copy(dsts[2 * p + 1][64:128, sl],
                                      pss[j][64:128, :])

        def emit_proj_v(mc):
            ps = fpool.tile([128, HPC * DK], F32, tag="pf", name="ps_v")
            for kc in range(KC):
                nc.tensor.matmul(
                    ps[:],
                    vT_sb[:, kc, mc * 128:(mc + 1) * 128],
                    wv_sb[:, kc, :],
                    start=(kc == 0),
                    stop=(kc == KC - 1),
                )
            nc.scalar.copy(
                vh_sb[:, mc, :, 0:DK],
                ps[:].rearrange("p (h k) -> p h k", k=DK),
            )

        def emit_outproj_half(qi, j, pool, tag, eng):
            po = pool.tile([128, 512], F32, tag=tag, name="po")
            for p in range(PAIRS):
                nc.tensor.matmul(
                    po[:],
                    outT2_sb[p][:, qi * 128:(qi + 1) * 128],
                    wo_sb[:, p, j * 512:(j + 1) * 512],
                    start=(p == 0),
                    stop=(p == PAIRS - 1),
                )
            so = ostage.tile([128, 512], BF16, tag="so", name="so")
            if eng == 0:
                nc.vector.tensor_copy(so[:], po[:])
            else:
                nc.scalar.copy(so[:], po[:])
            nc.sync.dma_start(
                outa[qi * 128:(qi + 1) * 128, j * 512:(j + 1) * 512], so[:]
            )

        # ---- upfront PE work: pair-0 kh (both sc) + qh sc0 ----------------
        emit_proj_qk(0, wk_sb, kT_sb, khp_sb, 0)
        emit_proj_qk(0, wk_sb, kT_sb, khp_sb, 1)
        emit_proj_qk(0, wq_sb, qT_sb, qhp_sb, 0)

        # Remaining projection groups drain as fills, in 2-kc quarters.
        def proj_quarters(p, w_sb, src, dsts, sc):
            state = {}

            def quarter(i, state=state, p=p, w_sb=w_sb, src=src, dsts=dsts,
                        sc=sc):
                if i == 0:
                    state["ps"] = fpool.tile([128, 1024], F32, tag="pf",
                                             name="ps_fq")
                ps = state["ps"]
                for kc in range(2 * i, 2 * i + 2):
                    for j in range(2):
                        nc.tensor.matmul(
                            ps[:, j * 512:(j + 1) * 512],
                            w_sb[:, kc, p * 128:(p + 1) * 128],
                            src[:, kc, sc * 1024 + j * 512: sc * 1024 + (j + 1) * 512],
                            start=(kc == 0),
                            stop=(kc == KC - 1),
                        )
                if i == 3:
                    # ACT does these copies: DVE is saturated by exp j1-halves
                    sl = slice(sc * 1024, (sc + 1) * 1024)
                    nc.scalar.copy(dsts[2 * p][0:64, sl], ps[0:64, :])
                    nc.scalar.copy(dsts[2 * p + 1][64:128, sl], ps[64:128, :])

            return [lambda i=i: quarter(i) for i in range(4)]

        def vproj_item(mp):
            def item(mp=mp):
                emit_proj_v(2 * mp)
                emit_proj_v(2 * mp + 1)
            return item

        # deadline-ordered fills: vh pairs first (unit 0 defers attn@v by 8
        # chunks), pair-0 qh sc1 before unit 2, pair-1 k/q before unit 4,
        # pair-1 qh sc1 before unit 6.
        fill_queue = (
            [vproj_item(mp) for mp in range(MC // 2)]
            + proj_quarters(0, wq_sb, qT_sb, qhp_sb, 1)
            + proj_quarters(1, wk_sb, kT_sb, khp_sb, 0)
            + proj_quarters(1, wk_sb, kT_sb, khp_sb, 1)
            + proj_quarters(1, wq_sb, qT_sb, qhp_sb, 0)
            + proj_quarters(1, wq_sb, qT_sb, qhp_sb, 1)
        )

        # ---- attention ----------------------------------------------------
        def emit_av(st, mc):
            h = st["h"]
            if st["pout"] is None:
                st["pout"] = popool.tile([65, 1024], F32, tag="po",
                                         name="pout")
            for j in range(2):
                nc.tensor.matmul(
                    st["pout"][:, j * 512:(j + 1) * 512],
                    vh_sb[:, mc, h, :],
                    st["at"][mc][j][:],
                    start=(mc == 0),
                    stop=(mc == MC - 1),
                )
            del st["at"][mc]

        def emit_dance(st):
            p, hh, qc = st["p"], st["hh"], st["qc"]
            hlo, hhi = hh * 64, hh * 64 + 64
            pout = st["pout"]
            sums = dance.tile([1, 1024], F32, tag="sums", name="sums")
            nc.vector.tensor_copy(sums[:], pout[64:65, :])
            rcp32 = dance.tile([1, 1024], F32, tag="rcp32", name="rcp32")
            nc.vector.reciprocal_approx_fast(rcp32[:], sums[:])
            rcpb = dance.tile([64, 1024], F32, tag="rcpb", name="rcpb")
            nc.gpsimd.partition_broadcast(rcpb[:], rcp32[:])
            nc.vector.tensor_tensor(
                outT2_sb[p][hlo:hhi, qc * 1024:(qc + 1) * 1024],
                pout[0:64, :],
                rcpb[:],
                mybir.AluOpType.mult,
            )

        def attention_unit(h, qc, fills, carry, lag):
            p, hh = h // 2, h % 2
            kh = khp_sb[h]
            qh = qhp_sb[h]
            st = {"h": h, "p": p, "hh": hh, "qc": qc, "at": {}, "pout": None}
            for mc in range(MC):
                if carry and mc >= 1:
                    carry.pop(0)()
                ps0 = ppool.tile([128, 512], F32, tag="pp", name="ps0")
                ps8 = ppool.tile([128, 512], F32, tag="pp", name="ps8")
                nc.tensor.matmul(
                    ps0[:],
                    kh[:, mc * 128:(mc + 1) * 128],
                    qh[:, qc * 1024: qc * 1024 + 512],
                    start=True, stop=True,
                )
                nc.tensor.matmul(
                    ps8[:],
                    kh[:, mc * 128:(mc + 1) * 128],
                    qh[:, qc * 1024 + 512: qc * 1024 + 1024],
                    start=True, stop=True,
                )
                # separate tiles per engine: a shared tile would serialize
                # ACT and DVE on the tile-level write dependency
                at0 = att.tile([128, 512], BF16, tag="at0", name="at0")
                at8 = att.tile([128, 512], BF16, tag="at8", name="at8")
                st["at"][mc] = (at0, at8)
                nc.scalar.activation(at0[:], ps0[:], AFT.Exp, scale=0.125)
                nc.vector._custom_dve(EXP16, out=at8[:], in0=ps8[:],
                                      s0=EXPC0, s1=EXPC1)
                if mc >= lag:
                    emit_av(st, mc - lag)
                if fills and mc % 2 == 0 and mc < 14:
                    fills.pop(0)()
            return [lambda m=m: emit_av(st, m) for m in range(MC - lag, MC)] + [
                lambda: emit_dance(st)
            ]



        units = [(0, 0, 0), (0, 1, 0), (0, 0, 1), (0, 1, 1),
                 (1, 0, 0), (1, 1, 0), (1, 0, 1), (1, 1, 1)]
        carry = []
        for u, (p, hh, qc) in enumerate(units):
            h = 2 * p + hh
            lag = 8 if u == 0 else 2
            carry = attention_unit(h, qc, fill_queue, carry, lag)
        for f in carry:
            f()
        while fill_queue:
            fill_queue.pop(0)()
        # tail: all 32 outproj halves; rotate 3 psum pools + 2 staging
        # engines so PE streams back-to-back
        pools = [(ppool, "pp"), (popool, "po"), (fpool, "pf")]
        tail = [(qi, j) for qi in range(16) for j in range(2)]
        for i, (qi, j) in enumerate(tail):
            pool, tag = pools[i % 3]
            emit_outproj_half(qi, j, pool, tag, eng=i % 2)


def build_program():
    nc = bacc.Bacc(
        "TRN2",
        target_bir_lowering=False,
        debug=False,
        enable_asserts=False,
        num_devices=N_CORES,
    )
    qT = nc.dram_tensor("qT", [D, S], BF16, kind="ExternalInput")
    kT = nc.dram_tensor("kT", [D, S], BF16, kind="ExternalInput")
    vT = nc.dram_tensor("vT", [D, S], BF16, kind="ExternalInput")
    wq = nc.dram_tensor("wq", [D, HPC * DK], BF16, kind="ExternalInput")
    wk = nc.dram_tensor("wk", [D, HPC * DK], BF16, kind="ExternalInput")
    wv = nc.dram_tensor("wv", [D, HPC * DK], BF16, kind="ExternalInput")
    wo = nc.dram_tensor("wo", [HPC * DK, D], BF16, kind="ExternalInput")
    out = nc.dram_tensor("out", [S, D], BF16, kind="ExternalOutput")
    with tile.TileContext(nc) as tc:
        _emit(tc, qT, kT, vT, wq, wk, wv, wo, out)
    nc.compile()
    return nc


def _get_program():
    if "nc" not in _COMPILED:
        _COMPILED["nc"] = build_program()
    return _COMPILED["nc"]


def make_in_maps(q, k, v, Wq, Wk, Wv, Wo):
    """Shard FULL fp32 inputs into per-core bf16 input maps."""
    q, k, v = (np.asarray(x, np.float32) for x in (q, k, v))
    Wq, Wk, Wv, Wo = (np.asarray(x, np.float32) for x in (Wq, Wk, Wv, Wo))
    qT = [np.ascontiguousarray(q[b].T).astype(BF16_NP) for b in range(B)]
    kT = [np.ascontiguousarray(k[b].T).astype(BF16_NP) for b in range(B)]
    vT = [np.ascontiguousarray(v[b].T).astype(BF16_NP) for b in range(B)]
    in_maps = []
    for c in range(N_CORES):
        b, g = divmod(c, N_CORES // B)
        heads = range(HPC * g, HPC * g + HPC)
        wq_c = np.concatenate([Wq[h] for h in heads], axis=1).astype(BF16_NP)
        wk_c = np.concatenate([Wk[h] for h in heads], axis=1).astype(BF16_NP)
        wv_c = np.concatenate([Wv[h] for h in heads], axis=1).astype(BF16_NP)
        wo_c = np.concatenate(
            [Wo[h * DK:(h + 1) * DK] for h in heads], axis=0
        ).astype(BF16_NP)
        in_maps.append({
            "qT": qT[b], "kT": kT[b], "vT": vT[b],
            "wq": np.ascontiguousarray(wq_c),
            "wk": np.ascontiguousarray(wk_c),
            "wv": np.ascontiguousarray(wv_c),
            "wo": np.ascontiguousarray(wo_c),
        })
    return in_maps


def run_on_hw(in_maps, trace=False):
    nc = _get_program()
    return bass_utils.run_bass_kernel_spmd(
        nc, in_maps, list(range(N_CORES)), trace=trace
    )


def kernel(q, k, v, Wq, Wk, Wv, Wo, bo):
    in_maps = make_in_maps(q, k, v, Wq, Wk, Wv, Wo)
    res = run_on_hw(in_maps)
    bo = np.asarray(bo, np.float32)
    parts = [np.asarray(r["out"], np.float32) for r in res.results]
    out = np.empty((B, S, D), np.float32)
    per_b = N_CORES // B
    for b in range(B):
        out[b] = np.sum(parts[b * per_b:(b + 1) * per_b], axis=0) + bo
    return out


# revision 50
# speedup vs baseline: 1.0126x; 1.0126x over previous
"""Multi-head attention (B=2, S=2048, D=1024, H=16, DK=64) on 8 Trainium2 cores.

Sharding: 8 cores x (1 batch, 4 heads) each.  Core c handles batch c//4,
heads [4*(c%4) : 4*(c%4)+4].  Each core computes its heads' slice of the
output projection (rows of Wo for its heads); the host sums the 4 partial
outputs per batch (fp32) and adds the bias.

Per-core dataflow (all matmuls bf16, PSUM fp32):
  - projections as in the classic layout: qhT/khT [128=head-pair, S] via PE;
    swap tiles (SBUF->SBUF DMA) duplicate each head's 64 partitions into the
    opposite half so scores can use PE ROW TILING: the two 512-wide j-halves
    of every scores chunk run CONCURRENTLY on sub-arrays T0 (partitions
    0-63) and T8 (64-127), halving effective scores time at K=64.
  - exp split across TWO engines per m-chunk: j0 half on ACT (table exp),
    j1 half on a custom fused DVE op (1 + w(c0 + c1 w))^16, w = z/128,
    equal throughput, both writing the same bf16 attnT tile.
  - scores psum is [128,512] (1 bank) double-buffered per engine; pout
    [65,1024] double-buffered; one [128,1024] fill slot => exactly 8 banks.
  - v-projection runs as fill work (vT arrives 3rd in the DMA order); the
    first attention unit defers attn@v by 8 m-chunks to cover the vT DMA.
  - output projection bf16, staged bf16 (Pool engine) and summed on host.
"""

import numpy as np
import ml_dtypes
from contextlib import ExitStack

import concourse.bass as bass
import concourse.tile as tile
from concourse import bacc, mybir
from concourse import bass_utils
from concourse import dve_ops
from concourse.dve_spec import Spec, Src0, One, C0, C1, sq, lower
from concourse.dve_uop import DveOpSpec
from concourse.dve_table_gen import dve_ver_for

B, S, D, H, DK = 2, 2048, 1024, 16, 64
N_CORES = 8
HPC = 4            # heads per core
PAIRS = HPC // 2   # head pairs per core
KC = D // 128      # contraction chunks over D
MC = S // 128      # m (key) chunks
F32 = mybir.dt.float32
BF16 = mybir.dt.bfloat16
BF16_NP = ml_dtypes.bfloat16

_COMPILED = {}

# ---- custom DVE op: exp(z/8) = p(z/128)^16, p = 1 + w(c0 + c1 w) ----------


def _fit_exp_coeffs():
    w = np.linspace(-0.2, 0.2, 8001)
    a_mat = np.stack([w, w * w], axis=1)
    t = np.exp(w)
    rw = 1.0 / t  # minimize relative error
    coef, *_ = np.linalg.lstsq(a_mat * rw[:, None], (t - 1.0) * rw, rcond=None)
    return [float(c) for c in coef]


_A1, _A2 = _fit_exp_coeffs()
EXPC0 = _A1 / 128.0
EXPC1 = _A2 / (128.0 ** 2)


def _exp16_ref(in0, in1, c0, c1, c2):
    x = np.asarray(in0, np.float32)
    p = (1.0 + x * (c0 + x * c1)).astype(np.float32)
    for _ in range(4):
        p = (p * p).astype(np.float32)
    return p


def _register_exp16():
    name = "EXP16_ANT"
    if name in dve_ops._SUB_OPCODE_FOR_NAME:
        for op in dve_ops.OPS:
            if op.name == name:
                return op
    body = sq(sq(sq(sq(One + Src0 * (C0 + Src0 * C1)))))
    spec = Spec(body=body, reference=_exp16_ref)
    row = max(dve_ops._SUB_OPCODE_FOR_NAME.values()) + 1
    dve_ops._SUB_OPCODE_FOR_NAME[name] = row
    ver = dve_ver_for("TRN2")
    tmp = DveOpSpec(name=name, opcode=row, uops=lower(spec, ver=ver), rd1_en=False)
    op = dve_ops.DveOp(name, spec, subdim=False, uops_sha={ver: tmp.sha(ver)})
    dve_ops.OPS.append(op)
    dve_ops.CUSTOM_DVE_SPECS[name] = spec
    return op


EXP16 = _register_exp16()


def _emit(tc, qT, kT, vT, wq, wk, wv, wo, out_dram):
    nc = tc.nc
    AFT = mybir.ActivationFunctionType
    qTa, kTa, vTa = qT.ap(), kT.ap(), vT.ap()
    wqa, wka, wva, woa = wq.ap(), wk.ap(), wv.ap(), wo.ap()
    outa = out_dram.ap()

    with ExitStack() as ctx:
        big = ctx.enter_context(tc.tile_pool(name="big", bufs=1))
        att = ctx.enter_context(tc.tile_pool(name="att", bufs=12))
        dance = ctx.enter_context(tc.tile_pool(name="dance", bufs=1))
        ostage = ctx.enter_context(tc.tile_pool(name="ostage", bufs=4))
        # PSUM (8 banks): scores 2x[128,512] = 2, pout 2x[65,1024] = 4,
        # fill slot 1x[128,1024] = 2.
        ppool = ctx.enter_context(tc.tile_pool(name="psum", bufs=2, space="PSUM"))
        popool = ctx.enter_context(tc.tile_pool(name="psum_o", bufs=2, space="PSUM"))
        fpool = ctx.enter_context(tc.tile_pool(name="psum_f", bufs=1, space="PSUM"))

        # ---- weights first (small), then chunked inputs -------------------
        wq_sb = big.tile([128, KC, HPC * DK], BF16, tag="wq")
        wk_sb = big.tile([128, KC, HPC * DK], BF16, tag="wk")
        wv_sb = big.tile([128, KC, HPC * DK], BF16, tag="wv")
        wo_sb = big.tile([128, PAIRS, D], BF16, tag="wo")
        nc.sync.dma_start(wk_sb[:], wka.rearrange("(c p) n -> p c n", p=128))
        nc.sync.dma_start(wq_sb[:], wqa.rearrange("(c p) n -> p c n", p=128))

        # warm the ACT exp table during the DMA phase
        warm_sb = big.tile([1, 64], BF16, tag="warm")
        nc.vector.memset(warm_sb[:], 1.0)
        nc.scalar.activation(warm_sb[:], warm_sb[:], AFT.Exp)

        # DMA order: kT -> qT(qc0 half) -> vT -> qT(qc1 half) -> wv, wo.
        kT_sb = big.tile([128, KC, S], BF16, tag="kT")
        qT_sb = big.tile([128, KC, S], BF16, tag="qT")
        vT_sb = big.tile([128, KC, S], BF16, tag="vT")
        kTr = kTa.rearrange("(c p) s -> p c s", p=128)
        qTr = qTa.rearrange("(c p) s -> p c s", p=128)
        nc.sync.dma_start(kT_sb[:], kTr)
        nc.sync.dma_start(qT_sb[:, :, 0:1024], qTr[:, :, 0:1024])
        nc.sync.dma_start(wv_sb[:], wva.rearrange("(c p) n -> p c n", p=128))
        nc.sync.dma_start(vT_sb[:], vTa.rearrange("(c p) s -> p c s", p=128))
        nc.sync.dma_start(qT_sb[:, :, 1024:2048], qTr[:, :, 1024:2048])
        nc.sync.dma_start(wo_sb[:], woa.rearrange("(c p) d -> p c d", p=128))

        # vh with a ones column per (m-chunk, head): [128, MC, HPC, 65]
        vh_sb = big.tile([128, MC, HPC, DK + 1], BF16, tag="vh")
        nc.vector.memset(vh_sb[:], 1.0)

        # per-head qh/kh tiles with the OTHER head's partition half zeroed,
        # so scores contract over the full 128 partitions: every matmul in
        # the kernel is then (128,128)-mode -> no PE tiling-mode switches.
        qhp_sb = [big.tile([128, S], BF16, tag=f"qhp{h}", name=f"qhp{h}")
                  for h in range(HPC)]
        khp_sb = [big.tile([128, S], BF16, tag=f"khp{h}", name=f"khp{h}")
                  for h in range(HPC)]
        for h in range(HPC):
            nc.gpsimd.memset(qhp_sb[h][:], 0.0)
            nc.gpsimd.memset(khp_sb[h][:], 0.0)
        outT2_sb = [big.tile([128, S], BF16, tag=f"o2{p}", name=f"o2{p}")
                    for p in range(PAIRS)]

        def emit_proj_qk(p, w_sb, src, dsts, sc):
            ps = popool.tile([128, 1024], F32, tag="po", name="ps_proj")
            for kc in range(KC):
                for j in range(2):
                    nc.tensor.matmul(
                        ps[:, j * 512:(j + 1) * 512],
                        w_sb[:, kc, p * 128:(p + 1) * 128],
                        src[:, kc, sc * 1024 + j * 512: sc * 1024 + (j + 1) * 512],
                        start=(kc == 0),
                        stop=(kc == KC - 1),
                    )
            sl = slice(sc * 1024, (sc + 1) * 1024)
            nc.vector.tensor_copy(dsts[2 * p][0:64, sl], ps[0:64, :])
            nc.vector.tensor_copy(dsts[2 * p + 1][64:128, sl], ps[64:128, :])

        def emit_proj_v(mc):
            ps = fpool.tile([128, HPC * DK], F32, tag="pf", name="ps_v")
            for kc in range(KC):
                nc.tensor.matmul(
                    ps[:],
                    vT_sb[:, kc, mc * 128:(mc + 1) * 128],
                    wv_sb[:, kc, :],
                    start=(kc == 0),
                    stop=(kc == KC - 1),
                )
            nc.scalar.copy(
                vh_sb[:, mc, :, 0:DK],
                ps[:].rearrange("p (h k) -> p h k", k=DK),
            )

        def emit_outproj_half(qi, j, pool, tag, eng):
            po = pool.tile([128, 512], F32, tag=tag, name="po")
            for p in range(PAIRS):
                nc.tensor.matmul(
                    po[:],
                    outT2_sb[p][:, qi * 128:(qi + 1) * 128],
                    wo_sb[:, p, j * 512:(j + 1) * 512],
                    start=(p == 0),
                    stop=(p == PAIRS - 1),
                )
            so = ostage.tile([128, 512], BF16, tag="so", name="so")
            if eng == 0:
                nc.vector.tensor_copy(so[:], po[:])
            else:
                nc.scalar.copy(so[:], po[:])
            nc.sync.dma_start(
                outa[qi * 128:(qi + 1) * 128, j * 512:(j + 1) * 512], so[:]
            )

        # ---- upfront PE work: pair-0 kh (both sc) + qh sc0 ----------------
        emit_proj_qk(0, wk_sb, kT_sb, khp_sb, 0)
        emit_proj_qk(0, wk_sb, kT_sb, khp_sb, 1)
        emit_proj_qk(0, wq_sb, qT_sb, qhp_sb, 0)

        # Remaining projection groups drain as fills, in 2-kc quarters.
        def proj_quarters(p, w_sb, src, dsts, sc):
            state = {}

            def quarter(i, state=state, p=p, w_sb=w_sb, src=src, dsts=dsts,
                        sc=sc):
                if i == 0:
                    state["ps"] = fpool.tile([128, 1024], F32, tag="pf",
                                             name="ps_fq")
                ps = state["ps"]
                for kc in range(2 * i, 2 * i + 2):
                    for j in range(2):
                        nc.tensor.matmul(
                            ps[:, j * 512:(j + 1) * 512],
                            w_sb[:, kc, p * 128:(p + 1) * 128],
                            src[:, kc, sc * 1024 + j * 512: sc * 1024 + (j + 1) * 512],
                            start=(kc == 0),
                            stop=(kc == KC - 1),
                        )
                if i == 3:
                    # ACT does these copies: DVE is saturated by exp j1-halves
                    sl = slice(sc * 1024, (sc + 1) * 1024)
                    nc.scalar.copy(dsts[2 * p][0:64, sl], ps[0:64, :])
                    nc.scalar.copy(dsts[2 * p + 1][64:128, sl], ps[64:128, :])

            return [lambda i=i: quarter(i) for i in range(4)]

        def vproj_item(mp):
            def item(mp=mp):
                emit_proj_v(2 * mp)
                emit_proj_v(2 * mp + 1)
            return item

        # deadline-ordered fills: vh pairs first (unit 0 defers attn@v by 8
        # chunks), pair-0 qh sc1 before unit 2, pair-1 k/q before unit 4,
        # pair-1 qh sc1 before unit 6.
        fill_queue = (
            [vproj_item(mp) for mp in range(MC // 2)]
            + proj_quarters(0, wq_sb, qT_sb, qhp_sb, 1)
            + proj_quarters(1, wk_sb, kT_sb, khp_sb, 0)
            + proj_quarters(1, wk_sb, kT_sb, khp_sb, 1)
            + proj_quarters(1, wq_sb, qT_sb, qhp_sb, 0)
            + proj_quarters(1, wq_sb, qT_sb, qhp_sb, 1)
        )

        # ---- attention ----------------------------------------------------
        def emit_av(st, mc):
            h = st["h"]
            if st["pout"] is None:
                st["pout"] = popool.tile([65, 1024], F32, tag="po",
                                         name="pout")
            for j in range(2):
                nc.tensor.matmul(
                    st["pout"][:, j * 512:(j + 1) * 512],
                    vh_sb[:, mc, h, :],
                    st["at"][mc][j][:],
                    start=(mc == 0),
                    stop=(mc == MC - 1),
                )
            del st["at"][mc]

        def emit_dance(st):
            p, hh, qc = st["p"], st["hh"], st["qc"]
            hlo, hhi = hh * 64, hh * 64 + 64
            pout = st["pout"]
            sums = dance.tile([1, 1024], F32, tag="sums", name="sums")
            nc.vector.tensor_copy(sums[:], pout[64:65, :])
            rcp32 = dance.tile([1, 1024], F32, tag="rcp32", name="rcp32")
            nc.vector.reciprocal_approx_fast(rcp32[:], sums[:])
            rcpb = dance.tile([64, 1024], F32, tag="rcpb", name="rcpb")
            nc.gpsimd.partition_broadcast(rcpb[:], rcp32[:])
            nc.vector.tensor_tensor(
                outT2_sb[p][hlo:hhi, qc * 1024:(qc + 1) * 1024],
                pout[0:64, :],
                rcpb[:],
                mybir.AluOpType.mult,
            )

        def attention_unit(h, qc, fills, carry, lag):
            p, hh = h // 2, h % 2
            kh = khp_sb[h]
            qh = qhp_sb[h]
            st = {"h": h, "p": p, "hh": hh, "qc": qc, "at": {}, "pout": None}
            for mc in range(MC):
                if carry and mc >= 1:
                    carry.pop(0)()
                ps0 = ppool.tile([128, 512], F32, tag="pp", name="ps0")
                ps8 = ppool.tile([128, 512], F32, tag="pp", name="ps8")
                nc.tensor.matmul(
                    ps0[:],
                    kh[:, mc * 128:(mc + 1) * 128],
                    qh[:, qc * 1024: qc * 1024 + 512],
                    start=True, stop=True,
                )
                nc.tensor.matmul(
                    ps8[:],
                    kh[:, mc * 128:(mc + 1) * 128],
                    qh[:, qc * 1024 + 512: qc * 1024 + 1024],
                    start=True, stop=True,
                )
                # separate tiles per engine: a shared tile would serialize
                # ACT and DVE on the tile-level write dependency
                at0 = att.tile([128, 512], BF16, tag="at0", name="at0")
                at8 = att.tile([128, 512], BF16, tag="at8", name="at8")
                st["at"][mc] = (at0, at8)
                nc.scalar.activation(at0[:], ps0[:], AFT.Exp, scale=0.125)
                nc.vector._custom_dve(EXP16, out=at8[:], in0=ps8[:],
                                      s0=EXPC0, s1=EXPC1)
                if mc >= lag:
                    emit_av(st, mc - lag)
                if fills and mc % 2 == 0 and mc < 14:
                    fills.pop(0)()
            return [lambda m=m: emit_av(st, m) for m in range(MC - lag, MC)] + [
                lambda: emit_dance(st)
            ]



        units = [(0, 0, 0), (0, 1, 0), (0, 0, 1), (0, 1, 1),
                 (1, 0, 0), (1, 1, 0), (1, 0, 1), (1, 1, 1)]
        carry = []
        for u, (p, hh, qc) in enumerate(units):
            h = 2 * p + hh
            lag = 8 if u == 0 else 2
            carry = attention_unit(h, qc, fill_queue, carry, lag)
        for f in carry:
            f()
        while fill_queue:
            fill_queue.pop(0)()
        # tail: all 32 outproj halves; rotate 3 psum pools + 2 staging
        # engines so PE streams back-to-back
        pools = [(ppool, "pp"), (popool, "po"), (fpool, "pf")]
        tail = [(qi, j) for qi in range(16) for j in range(2)]
        for i, (qi, j) in enumerate(tail):
            pool, tag = pools[i % 3]
            emit_outproj_half(qi, j, pool, tag, eng=i % 2)


def build_program():
    nc = bacc.Bacc(
        "TRN2",
        target_bir_lowering=False,
        debug=False,
        enable_asserts=False,
        num_devices=N_CORES,
    )
    qT = nc.dram_tensor("qT", [D, S], BF16, kind="ExternalInput")
    kT = nc.dram_tensor("kT", [D, S], BF16, kind="ExternalInput")
    vT = nc.dram_tensor("vT", [D, S], BF16, kind="ExternalInput")
    wq = nc.dram_tensor("wq", [D, HPC * DK], BF16, kind="ExternalInput")
    wk = nc.dram_tensor("wk", [D, HPC * DK], BF16, kind="ExternalInput")
    wv = nc.dram_tensor("wv", [D, HPC * DK], BF16, kind="ExternalInput")
    wo = nc.dram_tensor("wo", [HPC * DK, D], BF16, kind="ExternalInput")
    out = nc.dram_tensor("out", [S, D], BF16, kind="ExternalOutput")
    with tile.TileContext(nc) as tc:
        _emit(tc, qT, kT, vT, wq, wk, wv, wo, out)
    nc.compile()
    return nc


def _get_program():
    if "nc" not in _COMPILED:
        _COMPILED["nc"] = build_program()
    return _COMPILED["nc"]


def make_in_maps(q, k, v, Wq, Wk, Wv, Wo):
    """Shard FULL fp32 inputs into per-core bf16 input maps."""
    q, k, v = (np.asarray(x, np.float32) for x in (q, k, v))
    Wq, Wk, Wv, Wo = (np.asarray(x, np.float32) for x in (Wq, Wk, Wv, Wo))
    qT = [np.ascontiguousarray(q[b].T).astype(BF16_NP) for b in range(B)]
    kT = [np.ascontiguousarray(k[b].T).astype(BF16_NP) for b in range(B)]
    vT = [np.ascontiguousarray(v[b].T).astype(BF16_NP) for b in range(B)]
    in_maps = []
    for c in range(N_CORES):
        b, g = divmod(c, N_CORES // B)
        heads = range(HPC * g, HPC * g + HPC)
        wq_c = np.concatenate([Wq[h] for h in heads], axis=1).astype(BF16_NP)
        wk_c = np.concatenate([Wk[h] for h in heads], axis=1).astype(BF16_NP)
        wv_c = np.concatenate([Wv[h] for h in heads], axis=1).astype(BF16_NP)
        wo_c = np.concatenate(
            [Wo[h * DK:(h + 1) * DK] for h in heads], axis=0
        ).astype(BF16_NP)
        in_maps.append({
            "qT": qT[b], "kT": kT[b], "vT": vT[b],
            "wq": np.ascontiguousarray(wq_c),
            "wk": np.ascontiguousarray(wk_c),
            "wv": np.ascontiguousarray(wv_c),
            "wo": np.ascontiguousarray(wo_c),
        })
    return in_maps


def run_on_hw(in_maps, trace=False):
    nc = _get_program()
    return bass_utils.run_bass_kernel_spmd(
        nc, in_maps, list(range(N_CORES)), trace=trace
    )


def kernel(q, k, v, Wq, Wk, Wv, Wo, bo):
    in_maps = make_in_maps(q, k, v, Wq, Wk, Wv, Wo)
    res = run_on_hw(in_maps)
    bo = np.asarray(bo, np.float32)
    parts = [np.asarray(r["out"], np.float32) for r in res.results]
    out = np.empty((B, S, D), np.float32)
    per_b = N_CORES // B
    for b in range(B):
        out[b] = np.sum(parts[b * per_b:(b + 1) * per_b], axis=0) + bo
    return out


# revision 55
# speedup vs baseline: 1.2304x; 1.2151x over previous
"""Multi-head attention (B=2, S=2048, D=1024, H=16, DK=64) on 8 Trainium2 cores.

Sharding: 8 cores x (1 batch, 4 heads) each.  Core c handles batch c//4,
heads [4*(c%4) : 4*(c%4)+4].  Each core computes its heads' slice of the
output projection (rows of Wo for its heads); the host sums the 4 partial
outputs per batch (fp32) and adds the bias.

Per-core dataflow (all matmuls bf16, PSUM fp32):
  - projections as in the classic layout: qhT/khT [128=head-pair, S] via PE;
    swap tiles (SBUF->SBUF DMA) duplicate each head's 64 partitions into the
    opposite half so scores can use PE ROW TILING: the two 512-wide j-halves
    of every scores chunk run CONCURRENTLY on sub-arrays T0 (partitions
    0-63) and T8 (64-127), halving effective scores time at K=64.
  - exp split across TWO engines per m-chunk: j0 half on ACT (table exp),
    j1 half on a custom fused DVE op (1 + w(c0 + c1 w))^16, w = z/128,
    equal throughput, both writing the same bf16 attnT tile.
  - scores psum is [128,512] (1 bank) double-buffered per engine; pout
    [65,1024] double-buffered; one [128,1024] fill slot => exactly 8 banks.
  - v-projection runs as fill work (vT arrives 3rd in the DMA order); the
    first attention unit defers attn@v by 8 m-chunks to cover the vT DMA.
  - output projection bf16, staged bf16 (Pool engine) and summed on host.
"""

import numpy as np
import ml_dtypes
from contextlib import ExitStack

import concourse.bass as bass
import concourse.tile as tile
from concourse import bacc, mybir
from concourse import bass_utils
from concourse import dve_ops
from concourse.dve_spec import Spec, Src0, One, C0, C1, sq, lower
from concourse.dve_uop import DveOpSpec
from concourse.dve_table_gen import dve_ver_for

B, S, D, H, DK = 2, 2048, 1024, 16, 64
N_CORES = 8
HPC = 4            # heads per core
PAIRS = HPC // 2   # head pairs per core
KC = D // 128      # contraction chunks over D
MC = S // 128      # m (key) chunks
F32 = mybir.dt.float32
BF16 = mybir.dt.bfloat16
BF16_NP = ml_dtypes.bfloat16

_COMPILED = {}

# ---- custom DVE op: exp(z/8) = p(z/128)^16, p = 1 + w(c0 + c1 w) ----------


def _fit_exp_coeffs():
    w = np.linspace(-0.2, 0.2, 8001)
    a_mat = np.stack([w, w * w], axis=1)
    t = np.exp(w)
    rw = 1.0 / t  # minimize relative error
    coef, *_ = np.linalg.lstsq(a_mat * rw[:, None], (t - 1.0) * rw, rcond=None)
    return [float(c) for c in coef]


_A1, _A2 = _fit_exp_coeffs()
EXPC0 = _A1 / 128.0
EXPC1 = _A2 / (128.0 ** 2)


def _exp16_ref(in0, in1, c0, c1, c2):
    x = np.asarray(in0, np.float32)
    p = (1.0 + x * (c0 + x * c1)).astype(np.float32)
    for _ in range(4):
        p = (p * p).astype(np.float32)
    return p


def _register_exp16():
    name = "EXP16_ANT"
    if name in dve_ops._SUB_OPCODE_FOR_NAME:
        for op in dve_ops.OPS:
            if op.name == name:
                return op
    body = sq(sq(sq(sq(One + Src0 * (C0 + Src0 * C1)))))
    spec = Spec(body=body, reference=_exp16_ref)
    row = max(dve_ops._SUB_OPCODE_FOR_NAME.values()) + 1
    dve_ops._SUB_OPCODE_FOR_NAME[name] = row
    ver = dve_ver_for("TRN2")
    tmp = DveOpSpec(name=name, opcode=row, uops=lower(spec, ver=ver), rd1_en=False)
    op = dve_ops.DveOp(name, spec, subdim=False, uops_sha={ver: tmp.sha(ver)})
    dve_ops.OPS.append(op)
    dve_ops.CUSTOM_DVE_SPECS[name] = spec
    return op


EXP16 = _register_exp16()


def _emit(tc, qT, kT, vT, wq, wk, wv, wo, out_dram):
    nc = tc.nc
    AFT = mybir.ActivationFunctionType
    qTa, kTa, vTa = qT.ap(), kT.ap(), vT.ap()
    wqa, wka, wva, woa = wq.ap(), wk.ap(), wv.ap(), wo.ap()
    outa = out_dram.ap()

    with ExitStack() as ctx:
        big = ctx.enter_context(tc.tile_pool(name="big", bufs=1))
        att = ctx.enter_context(tc.tile_pool(name="att", bufs=12))
        dance = ctx.enter_context(tc.tile_pool(name="dance", bufs=1))
        ostage = ctx.enter_context(tc.tile_pool(name="ostage", bufs=4))
        # PSUM (8 banks): scores 4x[128,512] = 4 (2-deep ping-pong per exp
        # engine, so exp[mc] overlaps scores[mc+1]), one pout [65,1024] = 2,
        # fill slot 1x[128,1024] = 2.
        ppool = ctx.enter_context(tc.tile_pool(name="psum", bufs=4, space="PSUM"))
        popool = ctx.enter_context(tc.tile_pool(name="psum_o", bufs=1, space="PSUM"))
        fpool = ctx.enter_context(tc.tile_pool(name="psum_f", bufs=1, space="PSUM"))

        # ---- weights first (small), then chunked inputs -------------------
        wq_sb = big.tile([128, KC, HPC * DK], BF16, tag="wq")
        wk_sb = big.tile([128, KC, HPC * DK], BF16, tag="wk")
        wv_sb = big.tile([128, KC, HPC * DK], BF16, tag="wv")
        wo_sb = big.tile([128, PAIRS, D], BF16, tag="wo")
        nc.sync.dma_start(wk_sb[:], wka.rearrange("(c p) n -> p c n", p=128))
        nc.sync.dma_start(wq_sb[:], wqa.rearrange("(c p) n -> p c n", p=128))

        # warm the ACT exp table during the DMA phase
        warm_sb = big.tile([1, 64], BF16, tag="warm")
        nc.vector.memset(warm_sb[:], 1.0)
        nc.scalar.activation(warm_sb[:], warm_sb[:], AFT.Exp)

        # DMA order: kT -> qT(qc0 half) -> vT -> qT(qc1 half) -> wv, wo.
        kT_sb = big.tile([128, KC, S], BF16, tag="kT")
        qT_sb = big.tile([128, KC, S], BF16, tag="qT")
        vT_sb = big.tile([128, KC, S], BF16, tag="vT")
        kTr = kTa.rearrange("(c p) s -> p c s", p=128)
        qTr = qTa.rearrange("(c p) s -> p c s", p=128)
        # vT lands BEFORE the q halves: v-projection fills then never stall,
        # which permits a uniform short attn@v lag across all units
        nc.sync.dma_start(kT_sb[:], kTr)
        nc.sync.dma_start(wv_sb[:], wva.rearrange("(c p) n -> p c n", p=128))
        nc.sync.dma_start(vT_sb[:], vTa.rearrange("(c p) s -> p c s", p=128))
        nc.sync.dma_start(qT_sb[:, :, 0:1024], qTr[:, :, 0:1024])
        nc.sync.dma_start(qT_sb[:, :, 1024:2048], qTr[:, :, 1024:2048])
        nc.sync.dma_start(wo_sb[:], woa.rearrange("(c p) d -> p c d", p=128))

        # vh with a ones column per (m-chunk, head): [128, MC, HPC, 65]
        vh_sb = big.tile([128, MC, HPC, DK + 1], BF16, tag="vh")
        nc.vector.memset(vh_sb[:], 1.0)

        # per-head qh/kh tiles with the OTHER head's partition half zeroed,
        # so scores contract over the full 128 partitions: every matmul in
        # the kernel is then (128,128)-mode -> no PE tiling-mode switches.
        qhp_sb = [big.tile([128, S], BF16, tag=f"qhp{h}", name=f"qhp{h}")
                  for h in range(HPC)]
        khp_sb = [big.tile([128, S], BF16, tag=f"khp{h}", name=f"khp{h}")
                  for h in range(HPC)]
        for h in range(HPC):
            nc.gpsimd.memset(qhp_sb[h][:], 0.0)
            nc.gpsimd.memset(khp_sb[h][:], 0.0)
        outT2_sb = [big.tile([128, S], BF16, tag=f"o2{p}", name=f"o2{p}")
                    for p in range(PAIRS)]

        def emit_proj_qk(p, w_sb, src, dsts, sc):
            ps = popool.tile([128, 1024], F32, tag="po", name="ps_proj")
            for kc in range(KC):
                for j in range(2):
                    nc.tensor.matmul(
                        ps[:, j * 512:(j + 1) * 512],
                        w_sb[:, kc, p * 128:(p + 1) * 128],
                        src[:, kc, sc * 1024 + j * 512: sc * 1024 + (j + 1) * 512],
                        start=(kc == 0),
                        stop=(kc == KC - 1),
                    )
            sl = slice(sc * 1024, (sc + 1) * 1024)
            nc.vector.tensor_copy(dsts[2 * p][0:64, sl], ps[0:64, :])
            nc.vector.tensor_copy(dsts[2 * p + 1][64:128, sl], ps[64:128, :])

        def emit_proj_v(mc):
            ps = fpool.tile([128, HPC * DK], F32, tag="pf", name="ps_v")
            for kc in range(KC):
                nc.tensor.matmul(
                    ps[:],
                    vT_sb[:, kc, mc * 128:(mc + 1) * 128],
                    wv_sb[:, kc, :],
                    start=(kc == 0),
                    stop=(kc == KC - 1),
                )
            nc.scalar.copy(
                vh_sb[:, mc, :, 0:DK],
                ps[:].rearrange("p (h k) -> p h k", k=DK),
            )

        def emit_outproj_half(qi, j, pool, tag, eng):
            po = pool.tile([128, 512], F32, tag=tag, name="po")
            for p in range(PAIRS):
                nc.tensor.matmul(
                    po[:],
                    outT2_sb[p][:, qi * 128:(qi + 1) * 128],
                    wo_sb[:, p, j * 512:(j + 1) * 512],
                    start=(p == 0),
                    stop=(p == PAIRS - 1),
                )
            so = ostage.tile([128, 512], BF16, tag="so", name="so")
            if eng == 0:
                nc.vector.tensor_copy(so[:], po[:])
            else:
                nc.scalar.copy(so[:], po[:])
            nc.sync.dma_start(
                outa[qi * 128:(qi + 1) * 128, j * 512:(j + 1) * 512], so[:]
            )

        # ---- upfront PE work: pair-0 kh (both sc) + qh sc0 ----------------
        emit_proj_qk(0, wk_sb, kT_sb, khp_sb, 0)
        emit_proj_qk(0, wk_sb, kT_sb, khp_sb, 1)
        emit_proj_qk(0, wq_sb, qT_sb, qhp_sb, 0)

        # Remaining projection groups drain as fills, in 2-kc quarters.
        def proj_quarters(p, w_sb, src, dsts, sc):
            state = {}

            def quarter(i, state=state, p=p, w_sb=w_sb, src=src, dsts=dsts,
                        sc=sc):
                if i == 0:
                    state["ps"] = fpool.tile([128, 1024], F32, tag="pf",
                                             name="ps_fq")
                ps = state["ps"]
                for kc in range(2 * i, 2 * i + 2):
                    for j in range(2):
                        nc.tensor.matmul(
                            ps[:, j * 512:(j + 1) * 512],
                            w_sb[:, kc, p * 128:(p + 1) * 128],
                            src[:, kc, sc * 1024 + j * 512: sc * 1024 + (j + 1) * 512],
                            start=(kc == 0),
                            stop=(kc == KC - 1),
                        )
                if i == 3:
                    # ACT does these copies: DVE is saturated by exp j1-halves
                    sl = slice(sc * 1024, (sc + 1) * 1024)
                    nc.scalar.copy(dsts[2 * p][0:64, sl], ps[0:64, :])
                    nc.scalar.copy(dsts[2 * p + 1][64:128, sl], ps[64:128, :])

            return [lambda i=i: quarter(i) for i in range(4)]

        def vproj_item(mp):
            def item(mp=mp):
                emit_proj_v(2 * mp)
                emit_proj_v(2 * mp + 1)
            return item

        # deadline-ordered fills: vh pairs first (unit 0 defers attn@v by 8
        # chunks), pair-0 qh sc1 before unit 2, pair-1 k/q before unit 4,
        # pair-1 qh sc1 before unit 6.
        fill_queue = (
            [vproj_item(mp) for mp in range(MC // 2)]
            + proj_quarters(0, wq_sb, qT_sb, qhp_sb, 1)
            + proj_quarters(1, wk_sb, kT_sb, khp_sb, 0)
            + proj_quarters(1, wk_sb, kT_sb, khp_sb, 1)
            + proj_quarters(1, wq_sb, qT_sb, qhp_sb, 0)
            + proj_quarters(1, wq_sb, qT_sb, qhp_sb, 1)
        )

        # ---- attention ----------------------------------------------------
        def emit_av(st, mc):
            h = st["h"]
            if st["pout"] is None:
                st["pout"] = popool.tile([65, 1024], F32, tag="po",
                                         name="pout")
            for j in range(2):
                nc.tensor.matmul(
                    st["pout"][:, j * 512:(j + 1) * 512],
                    vh_sb[:, mc, h, :],
                    st["at"][mc][j][:],
                    start=(mc == 0),
                    stop=(mc == MC - 1),
                )
            del st["at"][mc]

        def emit_dance(st):
            p, hh, qc = st["p"], st["hh"], st["qc"]
            hlo, hhi = hh * 64, hh * 64 + 64
            pout = st["pout"]
            sums = dance.tile([1, 1024], F32, tag="sums", name="sums")
            nc.vector.tensor_copy(sums[:], pout[64:65, :])
            rcp32 = dance.tile([1, 1024], F32, tag="rcp32", name="rcp32")
            nc.vector.reciprocal_approx_fast(rcp32[:], sums[:])
            rcpb = dance.tile([64, 1024], F32, tag="rcpb", name="rcpb")
            nc.gpsimd.partition_broadcast(rcpb[:], rcp32[:])
            nc.vector.tensor_tensor(
                outT2_sb[p][hlo:hhi, qc * 1024:(qc + 1) * 1024],
                pout[0:64, :],
                rcpb[:],
                mybir.AluOpType.mult,
            )

        def attention_unit(h, qc, fills, carry, lag):
            p, hh = h // 2, h % 2
            kh = khp_sb[h]
            qh = qhp_sb[h]
            st = {"h": h, "p": p, "hh": hh, "qc": qc, "at": {}, "pout": None}
            for mc in range(MC):
                # drain previous unit's tail fast (2/mc): av13,av14 @mc1,
                # av15+dance @mc2; own av0 @mc3 then safely reuses the
                # single pout slot
                if mc >= 1:
                    for _ in range(2):
                        if carry:
                            carry.pop(0)()
                ps0 = ppool.tile([128, 512], F32, tag="pp", name="ps0")
                ps8 = ppool.tile([128, 512], F32, tag="pp", name="ps8")
                nc.tensor.matmul(
                    ps0[:],
                    kh[:, mc * 128:(mc + 1) * 128],
                    qh[:, qc * 1024: qc * 1024 + 512],
                    start=True, stop=True,
                )
                nc.tensor.matmul(
                    ps8[:],
                    kh[:, mc * 128:(mc + 1) * 128],
                    qh[:, qc * 1024 + 512: qc * 1024 + 1024],
                    start=True, stop=True,
                )
                # separate tiles per engine: a shared tile would serialize
                # ACT and DVE on the tile-level write dependency
                at0 = att.tile([128, 512], BF16, tag="at0", name="at0")
                at8 = att.tile([128, 512], BF16, tag="at8", name="at8")
                st["at"][mc] = (at0, at8)
                nc.scalar.activation(at0[:], ps0[:], AFT.Exp, scale=0.125)
                nc.vector._custom_dve(EXP16, out=at8[:], in0=ps8[:],
                                      s0=EXPC0, s1=EXPC1)
                if mc >= lag:
                    emit_av(st, mc - lag)
                if fills and mc % 2 == 0 and mc < 14:
                    fills.pop(0)()
            return [lambda m=m: emit_av(st, m) for m in range(MC - lag, MC)] + [
                lambda: emit_dance(st)
            ]



        units = [(0, 0, 0), (0, 1, 0), (0, 0, 1), (0, 1, 1),
                 (1, 0, 0), (1, 1, 0), (1, 0, 1), (1, 1, 1)]
        carry = []
        for u, (p, hh, qc) in enumerate(units):
            h = 2 * p + hh
            carry = attention_unit(h, qc, fill_queue, carry, lag=3)
            if u == 6:
                # all qc0 dances have landed by u6-mc2: unit 7's fill slots
                # take the qc0 output projections
                for i, (qi, j) in enumerate((qi, j) for qi in range(8)
                                            for j in range(2)):
                    fill_queue.append(
                        lambda qi=qi, j=j, i=i: emit_outproj_half(
                            qi, j, fpool, "pf", eng=i % 2)
                    )
        for f in carry:
            f()
        while fill_queue:
            fill_queue.pop(0)()
        # tail: all 32 outproj halves; rotate 3 psum pools + 2 staging
        # engines so PE streams back-to-back
        pools = [(ppool, "pp"), (popool, "po"), (fpool, "pf")]
        tail = [(qi, j) for qi in range(8, 16) for j in range(2)]
        for i, (qi, j) in enumerate(tail):
            pool, tag = pools[i % 3]
            emit_outproj_half(qi, j, pool, tag, eng=i % 2)


def build_program():
    nc = bacc.Bacc(
        "TRN2",
        target_bir_lowering=False,
        debug=False,
        enable_asserts=False,
        num_devices=N_CORES,
    )
    qT = nc.dram_tensor("qT", [D, S], BF16, kind="ExternalInput")
    kT = nc.dram_tensor("kT", [D, S], BF16, kind="ExternalInput")
    vT = nc.dram_tensor("vT", [D, S], BF16, kind="ExternalInput")
    wq = nc.dram_tensor("wq", [D, HPC * DK], BF16, kind="ExternalInput")
    wk = nc.dram_tensor("wk", [D, HPC * DK], BF16, kind="ExternalInput")
    wv = nc.dram_tensor("wv", [D, HPC * DK], BF16, kind="ExternalInput")
    wo = nc.dram_tensor("wo", [HPC * DK, D], BF16, kind="ExternalInput")
    out = nc.dram_tensor("out", [S, D], BF16, kind="ExternalOutput")
    with tile.TileContext(nc) as tc:
        _emit(tc, qT, kT, vT, wq, wk, wv, wo, out)
    nc.compile()
    return nc


def _get_program():
    if "nc" not in _COMPILED:
        _COMPILED["nc"] = build_program()
    return _COMPILED["nc"]


def make_in_maps(q, k, v, Wq, Wk, Wv, Wo):
    """Shard FULL fp32 inputs into per-core bf16 input maps."""
    q, k, v = (np.asarray(x, np.float32) for x in (q, k, v))
    Wq, Wk, Wv, Wo = (np.asarray(x, np.float32) for x in (Wq, Wk, Wv, Wo))
    qT = [np.ascontiguousarray(q[b].T).astype(BF16_NP) for b in range(B)]
    kT = [np.ascontiguousarray(k[b].T).astype(BF16_NP) for b in range(B)]
    vT = [np.ascontiguousarray(v[b].T).astype(BF16_NP) for b in range(B)]
    in_maps = []
    for c in range(N_CORES):
        b, g = divmod(c, N_CORES // B)
        heads = range(HPC * g, HPC * g + HPC)
        wq_c = np.concatenate([Wq[h] for h in heads], axis=1).astype(BF16_NP)
        wk_c = np.concatenate([Wk[h] for h in heads], axis=1).astype(BF16_NP)
        wv_c = np.concatenate([Wv[h] for h in heads], axis=1).astype(BF16_NP)
        wo_c = np.concatenate(
            [Wo[h * DK:(h + 1) * DK] for h in heads], axis=0
        ).astype(BF16_NP)
        in_maps.append({
            "qT": qT[b], "kT": kT[b], "vT": vT[b],
            "wq": np.ascontiguousarray(wq_c),
            "wk": np.ascontiguousarray(wk_c),
            "wv": np.ascontiguousarray(wv_c),
            "wo": np.ascontiguousarray(wo_c),
        })
    return in_maps


def run_on_hw(in_maps, trace=False):
    nc = _get_program()
    return bass_utils.run_bass_kernel_spmd(
        nc, in_maps, list(range(N_CORES)), trace=trace
    )


def kernel(q, k, v, Wq, Wk, Wv, Wo, bo):
    in_maps = make_in_maps(q, k, v, Wq, Wk, Wv, Wo)
    res = run_on_hw(in_maps)
    bo = np.asarray(bo, np.float32)
    parts = [np.asarray(r["out"], np.float32) for r in res.results]
    out = np.empty((B, S, D), np.float32)
    per_b = N_CORES // B
    for b in range(B):
        out[b] = np.sum(parts[b * per_b:(b + 1) * per_b], axis=0) + bo
    return out


# revision 61
# speedup vs baseline: 1.2788x; 1.0394x over previous
"""Multi-head attention (B=2, S=2048, D=1024, H=16, DK=64) on 8 Trainium2 cores.

Sharding: 8 cores x (1 batch, 4 heads) each.  Core c handles batch c//4,
heads [4*(c%4) : 4*(c%4)+4].  Each core computes its heads' slice of the
output projection (rows of Wo for its heads); the host sums the 4 partial
outputs per batch (fp32) and adds the bias.

Per-core dataflow (all matmuls bf16, PSUM fp32):
  - projections as in the classic layout: qhT/khT [128=head-pair, S] via PE;
    swap tiles (SBUF->SBUF DMA) duplicate each head's 64 partitions into the
    opposite half so scores can use PE ROW TILING: the two 512-wide j-halves
    of every scores chunk run CONCURRENTLY on sub-arrays T0 (partitions
    0-63) and T8 (64-127), halving effective scores time at K=64.
  - exp split across TWO engines per m-chunk: j0 half on ACT (table exp),
    j1 half on a custom fused DVE op (1 + w(c0 + c1 w))^16, w = z/128,
    equal throughput, both writing the same bf16 attnT tile.
  - scores psum is [128,512] (1 bank) double-buffered per engine; pout
    [65,1024] double-buffered; one [128,1024] fill slot => exactly 8 banks.
  - v-projection runs as fill work (vT arrives 3rd in the DMA order); the
    first attention unit defers attn@v by 8 m-chunks to cover the vT DMA.
  - output projection bf16, staged bf16 (Pool engine) and summed on host.
"""

import numpy as np
import ml_dtypes
from contextlib import ExitStack

import concourse.bass as bass
import concourse.tile as tile
from concourse import bacc, mybir
from concourse import bass_utils
from concourse import dve_ops
from concourse.dve_spec import Spec, Src0, One, C0, C1, sq, lower
from concourse.dve_uop import DveOpSpec
from concourse.dve_table_gen import dve_ver_for

B, S, D, H, DK = 2, 2048, 1024, 16, 64
N_CORES = 8
HPC = 4            # heads per core
PAIRS = HPC // 2   # head pairs per core
KC = D // 128      # contraction chunks over D
MC = S // 128      # m (key) chunks
F32 = mybir.dt.float32
BF16 = mybir.dt.bfloat16
BF16_NP = ml_dtypes.bfloat16

_COMPILED = {}

# ---- custom DVE op: exp(z/8) = p(z/128)^16, p = 1 + w(c0 + c1 w) ----------


def _fit_exp_coeffs():
    w = np.linspace(-0.2, 0.2, 8001)
    a_mat = np.stack([w, w * w], axis=1)
    t = np.exp(w)
    rw = 1.0 / t  # minimize relative error
    coef, *_ = np.linalg.lstsq(a_mat * rw[:, None], (t - 1.0) * rw, rcond=None)
    return [float(c) for c in coef]


_A1, _A2 = _fit_exp_coeffs()
EXPC0 = _A1 / 128.0
EXPC1 = _A2 / (128.0 ** 2)


def _exp16_ref(in0, in1, c0, c1, c2):
    x = np.asarray(in0, np.float32)
    p = (1.0 + x * (c0 + x * c1)).astype(np.float32)
    for _ in range(4):
        p = (p * p).astype(np.float32)
    return p


def _register_exp16():
    name = "EXP16_ANT"
    if name in dve_ops._SUB_OPCODE_FOR_NAME:
        for op in dve_ops.OPS:
            if op.name == name:
                return op
    body = sq(sq(sq(sq(One + Src0 * (C0 + Src0 * C1)))))
    spec = Spec(body=body, reference=_exp16_ref)
    row = max(dve_ops._SUB_OPCODE_FOR_NAME.values()) + 1
    dve_ops._SUB_OPCODE_FOR_NAME[name] = row
    ver = dve_ver_for("TRN2")
    tmp = DveOpSpec(name=name, opcode=row, uops=lower(spec, ver=ver), rd1_en=False)
    op = dve_ops.DveOp(name, spec, subdim=False, uops_sha={ver: tmp.sha(ver)})
    dve_ops.OPS.append(op)
    dve_ops.CUSTOM_DVE_SPECS[name] = spec
    return op


EXP16 = _register_exp16()


def _emit(tc, qT, kT, vT, wq, wk, wv, wo, out_dram):
    nc = tc.nc
    AFT = mybir.ActivationFunctionType
    qTa, kTa, vTa = qT.ap(), kT.ap(), vT.ap()
    wqa, wka, wva, woa = wq.ap(), wk.ap(), wv.ap(), wo.ap()
    outa = out_dram.ap()

    with ExitStack() as ctx:
        big = ctx.enter_context(tc.tile_pool(name="big", bufs=1))
        att = ctx.enter_context(tc.tile_pool(name="att", bufs=12))
        dance = ctx.enter_context(tc.tile_pool(name="dance", bufs=1))
        ostage = ctx.enter_context(tc.tile_pool(name="ostage", bufs=4))
        # PSUM (8 banks): scores 4x[128,512] = 4 (2-deep ping-pong per exp
        # engine, so exp[mc] overlaps scores[mc+1]), one pout [65,1024] = 2,
        # fill slot 1x[128,1024] = 2.
        ppool = ctx.enter_context(tc.tile_pool(name="psum", bufs=4, space="PSUM"))
        popool = ctx.enter_context(tc.tile_pool(name="psum_o", bufs=1, space="PSUM"))
        fpool = ctx.enter_context(tc.tile_pool(name="psum_f", bufs=1, space="PSUM"))

        # ---- weights first (small), then chunked inputs -------------------
        wq_sb = big.tile([128, KC, HPC * DK], BF16, tag="wq")
        wk_sb = big.tile([128, KC, HPC * DK], BF16, tag="wk")
        wv_sb = big.tile([128, KC, HPC * DK], BF16, tag="wv")
        wo_sb = big.tile([128, PAIRS, D], BF16, tag="wo")
        nc.sync.dma_start(wk_sb[:], wka.rearrange("(c p) n -> p c n", p=128))
        nc.sync.dma_start(wq_sb[:], wqa.rearrange("(c p) n -> p c n", p=128))

        # warm the ACT exp table during the DMA phase
        warm_sb = big.tile([1, 64], BF16, tag="warm")
        nc.vector.memset(warm_sb[:], 1.0)
        nc.scalar.activation(warm_sb[:], warm_sb[:], AFT.Exp)

        # DMA order: kT -> qT(qc0 half) -> vT -> qT(qc1 half) -> wv, wo.
        kT_sb = big.tile([128, KC, S], BF16, tag="kT")
        qT_sb = big.tile([128, KC, S], BF16, tag="qT")
        vT_sb = big.tile([128, KC, S], BF16, tag="vT")
        kTr = kTa.rearrange("(c p) s -> p c s", p=128)
        qTr = qTa.rearrange("(c p) s -> p c s", p=128)
        # vT lands BEFORE the q halves: v-projection fills then never stall,
        # which permits a uniform short attn@v lag across all units
        nc.sync.dma_start(kT_sb[:], kTr)
        nc.sync.dma_start(wv_sb[:], wva.rearrange("(c p) n -> p c n", p=128))
        nc.sync.dma_start(vT_sb[:], vTa.rearrange("(c p) s -> p c s", p=128))
        # qc0 half arrives in per-kc chunks so the q-projection can track
        # the DMA instead of waiting for the whole half
        for kc in range(KC):
            nc.sync.dma_start(qT_sb[:, kc, 0:1024], qTr[:, kc, 0:1024])
        nc.sync.dma_start(qT_sb[:, :, 1024:2048], qTr[:, :, 1024:2048])
        nc.sync.dma_start(wo_sb[:], woa.rearrange("(c p) d -> p c d", p=128))

        # vh with a ones column per (m-chunk, head): [128, MC, HPC, 65]
        vh_sb = big.tile([128, MC, HPC, DK + 1], BF16, tag="vh")
        nc.vector.memset(vh_sb[:], 1.0)

        # per-head qh/kh tiles with the OTHER head's partition half zeroed,
        # so scores contract over the full 128 partitions: every matmul in
        # the kernel is then (128,128)-mode -> no PE tiling-mode switches.
        qhp_sb = [big.tile([128, S], BF16, tag=f"qhp{h}", name=f"qhp{h}")
                  for h in range(HPC)]
        khp_sb = [big.tile([128, S], BF16, tag=f"khp{h}", name=f"khp{h}")
                  for h in range(HPC)]
        for h in range(HPC):
            nc.gpsimd.memset(qhp_sb[h][:], 0.0)
            nc.gpsimd.memset(khp_sb[h][:], 0.0)
        outT2_sb = [big.tile([128, S], BF16, tag=f"o2{p}", name=f"o2{p}")
                    for p in range(PAIRS)]

        def emit_proj_qk(p, w_sb, src, dsts, sc):
            ps = popool.tile([128, 1024], F32, tag="po", name="ps_proj")
            for kc in range(KC):
                for j in range(2):
                    nc.tensor.matmul(
                        ps[:, j * 512:(j + 1) * 512],
                        w_sb[:, kc, p * 128:(p + 1) * 128],
                        src[:, kc, sc * 1024 + j * 512: sc * 1024 + (j + 1) * 512],
                        start=(kc == 0),
                        stop=(kc == KC - 1),
                    )
            sl = slice(sc * 1024, (sc + 1) * 1024)
            nc.vector.tensor_copy(dsts[2 * p][0:64, sl], ps[0:64, :])
            nc.vector.tensor_copy(dsts[2 * p + 1][64:128, sl], ps[64:128, :])

        def emit_proj_v(mc):
            ps = fpool.tile([128, HPC * DK], F32, tag="pf", name="ps_v")
            for kc in range(KC):
                nc.tensor.matmul(
                    ps[:],
                    vT_sb[:, kc, mc * 128:(mc + 1) * 128],
                    wv_sb[:, kc, :],
                    start=(kc == 0),
                    stop=(kc == KC - 1),
                )
            nc.scalar.copy(
                vh_sb[:, mc, :, 0:DK],
                ps[:].rearrange("p (h k) -> p h k", k=DK),
            )

        def emit_outproj_half(qi, j, pool, tag, eng):
            po = pool.tile([128, 512], F32, tag=tag, name="po")
            for p in range(PAIRS):
                nc.tensor.matmul(
                    po[:],
                    outT2_sb[p][:, qi * 128:(qi + 1) * 128],
                    wo_sb[:, p, j * 512:(j + 1) * 512],
                    start=(p == 0),
                    stop=(p == PAIRS - 1),
                )
            so = ostage.tile([128, 512], BF16, tag="so", name="so")
            if eng == 0:
                nc.vector.tensor_copy(so[:], po[:])
            else:
                nc.scalar.copy(so[:], po[:])
            nc.sync.dma_start(
                outa[qi * 128:(qi + 1) * 128, j * 512:(j + 1) * 512], so[:]
            )

        # ---- upfront PE work: pair-0 kh (both sc) + qh sc0 ----------------
        emit_proj_qk(0, wk_sb, kT_sb, khp_sb, 0)
        emit_proj_qk(0, wk_sb, kT_sb, khp_sb, 1)
        # qh sc0 via two 512-wide psum chains on the scores pool, one kc at
        # a time, so each contraction step waits only its own qT chunk-DMA
        qps = [ppool.tile([128, 512], F32, tag="pp", name=f"qps{j}")
               for j in range(2)]
        for kc in range(KC):
            for j in range(2):
                nc.tensor.matmul(
                    qps[j][:],
                    wq_sb[:, kc, 0:128],
                    qT_sb[:, kc, j * 512:(j + 1) * 512],
                    start=(kc == 0),
                    stop=(kc == KC - 1),
                )
        for j in range(2):
            sl = slice(j * 512, (j + 1) * 512)
            nc.vector.tensor_copy(qhp_sb[0][0:64, sl], qps[j][0:64, :])
            nc.vector.tensor_copy(qhp_sb[1][64:128, sl], qps[j][64:128, :])

        # Remaining projection groups drain as fills, in 2-kc quarters.
        def proj_quarters(p, w_sb, src, dsts, sc):
            state = {}

            def quarter(i, state=state, p=p, w_sb=w_sb, src=src, dsts=dsts,
                        sc=sc):
                if i == 0:
                    state["ps"] = fpool.tile([128, 1024], F32, tag="pf",
                                             name="ps_fq")
                ps = state["ps"]
                for kc in range(2 * i, 2 * i + 2):
                    for j in range(2):
                        nc.tensor.matmul(
                            ps[:, j * 512:(j + 1) * 512],
                            w_sb[:, kc, p * 128:(p + 1) * 128],
                            src[:, kc, sc * 1024 + j * 512: sc * 1024 + (j + 1) * 512],
                            start=(kc == 0),
                            stop=(kc == KC - 1),
                        )
                if i == 3:
                    # ACT does these copies: DVE is saturated by exp j1-halves
                    sl = slice(sc * 1024, (sc + 1) * 1024)
                    nc.scalar.copy(dsts[2 * p][0:64, sl], ps[0:64, :])
                    nc.scalar.copy(dsts[2 * p + 1][64:128, sl], ps[64:128, :])

            return [lambda i=i: quarter(i) for i in range(4)]

        def vproj_item(mp):
            def item(mp=mp):
                emit_proj_v(2 * mp)
                emit_proj_v(2 * mp + 1)
            return item

        # deadline-ordered fills: vh pairs first (unit 0 defers attn@v by 8
        # chunks), pair-0 qh sc1 before unit 2, pair-1 k/q before unit 4,
        # pair-1 qh sc1 before unit 6.
        fill_queue = (
            [vproj_item(mp) for mp in range(MC // 2)]
            + proj_quarters(0, wq_sb, qT_sb, qhp_sb, 1)
            + proj_quarters(1, wk_sb, kT_sb, khp_sb, 0)
            + proj_quarters(1, wk_sb, kT_sb, khp_sb, 1)
            + proj_quarters(1, wq_sb, qT_sb, qhp_sb, 0)
            + proj_quarters(1, wq_sb, qT_sb, qhp_sb, 1)
        )

        # ---- attention ----------------------------------------------------
        def emit_av(st, mc):
            h = st["h"]
            if st["pout"] is None:
                st["pout"] = popool.tile([65, 1024], F32, tag="po",
                                         name="pout")
            for j in range(2):
                nc.tensor.matmul(
                    st["pout"][:, j * 512:(j + 1) * 512],
                    vh_sb[:, mc, h, :],
                    st["at"][mc][j][:],
                    start=(mc == 0),
                    stop=(mc == MC - 1),
                )
            del st["at"][mc]

        def emit_dance(st):
            p, hh, qc = st["p"], st["hh"], st["qc"]
            hlo, hhi = hh * 64, hh * 64 + 64
            pout = st["pout"]
            sums = dance.tile([1, 1024], F32, tag="sums", name="sums")
            nc.vector.tensor_copy(sums[:], pout[64:65, :])
            rcp32 = dance.tile([1, 1024], F32, tag="rcp32", name="rcp32")
            nc.vector.reciprocal_approx_fast(rcp32[:], sums[:])
            rcpb = dance.tile([64, 1024], F32, tag="rcpb", name="rcpb")
            nc.gpsimd.partition_broadcast(rcpb[:], rcp32[:])
            nc.vector.tensor_tensor(
                outT2_sb[p][hlo:hhi, qc * 1024:(qc + 1) * 1024],
                pout[0:64, :],
                rcpb[:],
                mybir.AluOpType.mult,
            )

        def attention_unit(h, qc, fills, carry, lag, extra=False):
            p, hh = h // 2, h % 2
            kh = khp_sb[h]
            qh = qhp_sb[h]
            st = {"h": h, "p": p, "hh": hh, "qc": qc, "at": {}, "pout": None}
            for mc in range(MC):
                # drain previous unit's tail fast (2/mc): av13,av14 @mc1,
                # av15+dance @mc2; own av0 @mc3 then safely reuses the
                # single pout slot
                if mc >= 1:
                    for _ in range(2):
                        if carry:
                            carry.pop(0)()
                ps0 = ppool.tile([128, 512], F32, tag="pp", name="ps0")
                ps8 = ppool.tile([128, 512], F32, tag="pp", name="ps8")
                nc.tensor.matmul(
                    ps0[:],
                    kh[:, mc * 128:(mc + 1) * 128],
                    qh[:, qc * 1024: qc * 1024 + 512],
                    start=True, stop=True,
                )
                nc.tensor.matmul(
                    ps8[:],
                    kh[:, mc * 128:(mc + 1) * 128],
                    qh[:, qc * 1024 + 512: qc * 1024 + 1024],
                    start=True, stop=True,
                )
                # separate tiles per engine: a shared tile would serialize
                # ACT and DVE on the tile-level write dependency
                at0 = att.tile([128, 512], BF16, tag="at0", name="at0")
                at8 = att.tile([128, 512], BF16, tag="at8", name="at8")
                st["at"][mc] = (at0, at8)
                nc.scalar.activation(at0[:], ps0[:], AFT.Exp, scale=0.125)
                if mc in (6, 12):
                    # DVE also carries the dance; hand it 2 of 16 tiles back
                    nc.scalar.activation(at8[:], ps8[:], AFT.Exp, scale=0.125)
                else:
                    nc.vector._custom_dve(EXP16, out=at8[:], in0=ps8[:],
                                          s0=EXPC0, s1=EXPC1)
                if mc >= lag:
                    emit_av(st, mc - lag)
                if fills and mc % 2 == 0 and mc < 14:
                    fills.pop(0)()
                    if extra and fills:
                        fills.pop(0)()
            return [lambda m=m: emit_av(st, m) for m in range(MC - lag, MC)] + [
                lambda: emit_dance(st)
            ]



        units = [(0, 0, 0), (0, 1, 0), (0, 0, 1), (0, 1, 1),
                 (1, 0, 0), (1, 1, 0), (1, 0, 1), (1, 1, 1)]
        carry = []
        for u, (p, hh, qc) in enumerate(units):
            h = 2 * p + hh
            carry = attention_unit(h, qc, fill_queue, carry, lag=3,
                                   extra=(u == 7))
            if u == 6:
                # all qc0 dances have landed by u6-mc2: unit 7's fill slots
                # take the qc0 output projections
                for i, (qi, j) in enumerate((qi, j) for qi in range(8)
                                            for j in range(2)):
                    fill_queue.append(
                        lambda qi=qi, j=j, i=i: emit_outproj_half(
                            qi, j, fpool, "pf", eng=i % 2)
                    )
        for f in carry:
            f()
        while fill_queue:
            fill_queue.pop(0)()
        # tail: all 32 outproj halves; rotate 3 psum pools + 2 staging
        # engines so PE streams back-to-back
        pools = [(ppool, "pp"), (popool, "po"), (fpool, "pf")]
        tail = [(qi, j) for qi in range(8, 16) for j in range(2)]
        for i, (qi, j) in enumerate(tail):
            pool, tag = pools[i % 3]
            emit_outproj_half(qi, j, pool, tag, eng=i % 2)


def build_program():
    nc = bacc.Bacc(
        "TRN2",
        target_bir_lowering=False,
        debug=False,
        enable_asserts=False,
        num_devices=N_CORES,
    )
    qT = nc.dram_tensor("qT", [D, S], BF16, kind="ExternalInput")
    kT = nc.dram_tensor("kT", [D, S], BF16, kind="ExternalInput")
    vT = nc.dram_tensor("vT", [D, S], BF16, kind="ExternalInput")
    wq = nc.dram_tensor("wq", [D, HPC * DK], BF16, kind="ExternalInput")
    wk = nc.dram_tensor("wk", [D, HPC * DK], BF16, kind="ExternalInput")
    wv = nc.dram_tensor("wv", [D, HPC * DK], BF16, kind="ExternalInput")
    wo = nc.dram_tensor("wo", [HPC * DK, D], BF16, kind="ExternalInput")
    out = nc.dram_tensor("out", [S, D], BF16, kind="ExternalOutput")
    with tile.TileContext(nc) as tc:
        _emit(tc, qT, kT, vT, wq, wk, wv, wo, out)
    nc.compile()
    return nc


def _get_program():
    if "nc" not in _COMPILED:
        _COMPILED["nc"] = build_program()
    return _COMPILED["nc"]


def make_in_maps(q, k, v, Wq, Wk, Wv, Wo):
    """Shard FULL fp32 inputs into per-core bf16 input maps."""
    q, k, v = (np.asarray(x, np.float32) for x in (q, k, v))
    Wq, Wk, Wv, Wo = (np.asarray(x, np.float32) for x in (Wq, Wk, Wv, Wo))
    qT = [np.ascontiguousarray(q[b].T).astype(BF16_NP) for b in range(B)]
    kT = [np.ascontiguousarray(k[b].T).astype(BF16_NP) for b in range(B)]
    vT = [np.ascontiguousarray(v[b].T).astype(BF16_NP) for b in range(B)]
    in_maps = []
    for c in range(N_CORES):
        b, g = divmod(c, N_CORES // B)
        heads = range(HPC * g, HPC * g + HPC)
        wq_c = np.concatenate([Wq[h] for h in heads], axis=1).astype(BF16_NP)
        wk_c = np.concatenate([Wk[h] for h in heads], axis=1).astype(BF16_NP)
        wv_c = np.concatenate([Wv[h] for h in heads], axis=1).astype(BF16_NP)
        wo_c = np.concatenate(
            [Wo[h * DK:(h + 1) * DK] for h in heads], axis=0
        ).astype(BF16_NP)
        in_maps.append({
            "qT": qT[b], "kT": kT[b], "vT": vT[b],
            "wq": np.ascontiguousarray(wq_c),
            "wk": np.ascontiguousarray(wk_c),
            "wv": np.ascontiguousarray(wv_c),
            "wo": np.ascontiguousarray(wo_c),
        })
    return in_maps


def run_on_hw(in_maps, trace=False):
    nc = _get_program()
    return bass_utils.run_bass_kernel_spmd(
        nc, in_maps, list(range(N_CORES)), trace=trace
    )


def kernel(q, k, v, Wq, Wk, Wv, Wo, bo):
    in_maps = make_in_maps(q, k, v, Wq, Wk, Wv, Wo)
    res = run_on_hw(in_maps)
    bo = np.asarray(bo, np.float32)
    parts = [np.asarray(r["out"], np.float32) for r in res.results]
    out = np.empty((B, S, D), np.float32)
    per_b = N_CORES // B
    for b in range(B):
        out[b] = np.sum(parts[b * per_b:(b + 1) * per_b], axis=0) + bo
    return out
